# revision 27
# baseline (speedup 1.0000x reference)
"""Trainium2 Bass kernel for nn_Attention_21122649161959.

RETRO-style causal self-attention block (RMSNorm -> q/kv proj -> RoPE ->
null-kv prepend -> causal masked softmax -> out proj) for
x [2, 2048, 1024], 16 heads of 64.

Sharding: 8 NeuronCores = (batch 2) x (4 head-groups of 4 heads).
Each core computes, for its batch b and heads [h0, h0+4):
    y_partial^T = Wout[h-slice]^T @ attn_out^T          [1024, 2048]
The host sums the 4 partials per batch, transposes, and adds the bias.

On-device layout is transposed (channels on partitions): xn^T is built with
PE transposes, projections run with the weight chunk as the stationary
operand, attention scores are computed as S^T[j, i] so softmax runs along
partitions via an appended ones-column in V (rowsum comes out of the same
matmul that computes attn@V). Softmax skips the row-max subtraction (logits
are bounded; padding mask enters as an additive per-key bias through the
Exp activation's bias port, causal mask multiplies E^T by 0/1 tiles on DVE).
All matmuls use float32r (full-rate fp32 PE mode, 11-bit mantissa).
"""

import sys

sys.path.insert(0, "/opt/trn_rl_repo")

from contextlib import ExitStack

import numpy as np

import concourse.bass as bass
import concourse.tile as tile
from concourse import bacc, mybir
from concourse.masks import make_identity

F32 = mybir.dt.float32
F32R = mybir.dt.float32r
BF16 = mybir.dt.bfloat16
AF = mybir.ActivationFunctionType

B, N, D = 2, 2048, 1024
H, DH = 16, 64
HPC = 4                 # heads per core
CPH = HPC * DH          # channels per core = 256
NCORES = 8
NJT = 17                # key tiles of 128 (2049 keys padded to 2176)
JPAD = NJT * 128
NCI = 4                 # query chunks of 512
NEG = -1e9
EPS = 1e-8

TRACE = False           # set by test harness for profiled runs
TRACE_KW = {}


def build_program():
    nc = bacc.Bacc(trn_type="TRN2", num_devices=NCORES)

    x_h = nc.dram_tensor("x", [N, D], F32, kind="ExternalInput")
    wq_h = nc.dram_tensor("wq", [D, CPH], F32R, kind="ExternalInput")
    wk_h = nc.dram_tensor("wk", [D, CPH], F32R, kind="ExternalInput")
    wv_h = nc.dram_tensor("wv", [D, CPH], F32R, kind="ExternalInput")
    wo_h = nc.dram_tensor("wo", [CPH, D], F32R, kind="ExternalInput")
    cos_h = nc.dram_tensor("cos2", [128, N], F32, kind="ExternalInput")
    sin_h = nc.dram_tensor("sin2", [128, N], F32, kind="ExternalInput")
    nk_h = nc.dram_tensor("nk", [2, 128], F32R, kind="ExternalInput")
    nv_h = nc.dram_tensor("nv", [1, CPH], F32R, kind="ExternalInput")
    mb_h = nc.dram_tensor("mb", [JPAD], F32, kind="ExternalInput")
    yt_h = nc.dram_tensor("yt", [D, N], F32, kind="ExternalOutput")

    with ExitStack() as ctx:
        tc = ctx.enter_context(tile.TileContext(nc))

        persist = ctx.enter_context(tc.tile_pool(name="persist", bufs=1))

        def single(shape, tag, pool=None, dt=F32):
            return (pool or persist).tile(shape, dt, tag=tag, name=tag)

        xnt = single([128, 8, N], "xnt", dt=F32R)
        qt = [single([128, N], f"qt{m}", dt=F32R) for m in range(2)]
        kt = [single([128, JPAD], f"kt{m}", dt=F32R) for m in range(2)]
        vsb = [single([128, HPC, DH + 1], f"v{j}", dt=F32R) for j in range(NJT)]
        mb_sb = single([128, NJT], "mb")

        for m in range(2):
            nc.sync.dma_start(
                out=kt[m][:, 0:1], in_=nk_h[m:m + 1, :].rearrange("o p -> p o")
            )
            nc.vector.memset(kt[m][:, 2049:JPAD].bitcast(F32), 0.0)
        nc.sync.dma_start(out=mb_sb, in_=mb_h.rearrange("(t p) -> p t", p=128))

        nc.vector.memset(vsb[16][:, :, 0:DH].bitcast(F32), 0.0)
        for j in range(NJT):
            nc.vector.memset(vsb[j][:, :, DH:DH + 1].bitcast(F32), 1.0)
        nc.sync.dma_start(
            out=vsb[0][0:1, :, 0:DH], in_=nv_h.rearrange("o (h d) -> o h d", h=HPC)
        )

        with tc.tile_pool(name="wpool", bufs=1) as wpool, \
             tc.tile_pool(name="trig", bufs=1) as trig, \
             tc.tile_pool(name="xin", bufs=3) as xin, \
             tc.tile_pool(name="stat", bufs=4) as stat, \
             tc.tile_pool(name="rope", bufs=2) as rope, \
             tc.tile_pool(name="psp", bufs=3, space="PSUM") as psp, \
             tc.tile_pool(name="pst", bufs=2, space="PSUM") as pst:

            wq_sb = [single([128, CPH], f"wq{k}", wpool, dt=F32R) for k in range(8)]
            wk_sb = [single([128, CPH], f"wk{k}", wpool, dt=F32R) for k in range(8)]
            wv_sb = [single([128, CPH], f"wv{k}", wpool, dt=F32R) for k in range(8)]
            cos_sb = single([128, N], "cos", trig)
            sin_sb = single([128, N], "sin", trig)
            ident = single([128, 128], "ident", trig)

            make_identity(nc, ident)

            for t in range(16):
                r0, r1 = t * 128, (t + 1) * 128
                xt = xin.tile([128, D], F32, tag="xt", name="xt")
                nc.sync.dma_start(out=xt, in_=x_h[r0:r1, :])
                sq = xin.tile([128, D], F32, tag="sq", name="sq")
                ssq = stat.tile([128, 1], F32, tag="ssq", name="ssq")
                nc.scalar.activation(out=sq, in_=xt, func=AF.Square, accum_out=ssq)
                nrm = stat.tile([128, 1], F32, tag="nrm", name="nrm")
                nc.scalar.activation(out=nrm, in_=ssq, func=AF.Sqrt, scale=1.0 / D)
                nc.vector.tensor_scalar_max(out=nrm, in0=nrm, scalar1=EPS)
                rs = stat.tile([128, 1], F32, tag="rs", name="rs")
                nc.vector.reciprocal(out=rs, in_=nrm)
                nc.vector.tensor_scalar_mul(out=sq, in0=xt, scalar1=rs)
                for k in range(8):
                    tps = pst.tile([128, 128], F32, tag="tps", name="tps")
                    nc.tensor.transpose(tps, sq[:, k * 128:(k + 1) * 128], ident)
                    nc.scalar.copy(out=xnt[:, k, r0:r1], in_=tps)

            for k in range(8):
                nc.sync.dma_start(out=wq_sb[k], in_=wq_h[k * 128:(k + 1) * 128, :])
                nc.sync.dma_start(out=wk_sb[k], in_=wk_h[k * 128:(k + 1) * 128, :])
                nc.sync.dma_start(out=wv_sb[k], in_=wv_h[k * 128:(k + 1) * 128, :])
            nc.sync.dma_start(out=cos_sb, in_=cos_h[:, :])
            nc.sync.dma_start(out=sin_sb, in_=sin_h[:, :])

            for mc in range(2):
                m0, m1 = mc * 128, (mc + 1) * 128
                for c in range(NCI):
                    s0, s1 = c * 512, (c + 1) * 512
                    for wsb, dst, off in ((wq_sb, qt, 0), (wk_sb, kt, 1)):
                        ps = psp.tile([128, 512], F32, tag="proj", name="ps")
                        for k in range(8):
                            nc.tensor.matmul(
                                ps,
                                wsb[k][:, m0:m1],
                                xnt[:, k, s0:s1],
                                start=(k == 0),
                                stop=(k == 7),
                            )
                        qraw = rope.tile([128, 512], F32, tag="qraw", name="qraw")
                        nc.vector.tensor_copy(out=qraw, in_=ps)
                        shuf = rope.tile([128, 512], F32, tag="shuf", name="shuf")
                        nc.vector.stream_shuffle(
                            out=shuf, in_=qraw, mask=[i ^ 1 for i in range(32)]
                        )
                        qc = rope.tile([128, 512], F32, tag="qc", name="qc")
                        nc.vector.tensor_mul(out=qc, in0=qraw, in1=cos_sb[:, s0:s1])
                        nc.gpsimd.tensor_tensor(
                            out=shuf, in0=shuf, in1=sin_sb[:, s0:s1],
                            op=mybir.AluOpType.mult,
                        )
                        nc.vector.tensor_add(
                            out=dst[mc][:, off + s0:off + s1], in0=qc, in1=shuf
                        )

            for j in range(16):
                ps = psp.tile([128, CPH], F32, tag="proj", name="psv")
                for k in range(8):
                    nc.tensor.matmul(
                        ps,
                        xnt[:, k, j * 128:(j + 1) * 128],
                        wv_sb[k],
                        start=(k == 0),
                        stop=(k == 7),
                    )
                vtmp = rope.tile([128, CPH], F32R, tag="shuf", name="vtmp")
                nc.scalar.copy(out=vtmp, in_=ps)
                nc.sync.dma_start(
                    out=vsb[j][1:128, :, 0:DH],
                    in_=vtmp[0:127, :].rearrange("p (h d) -> p h d", h=HPC),
                )
                nc.sync.dma_start(
                    out=vsb[j + 1][0:1, :, 0:DH],
                    in_=vtmp[127:128, :].rearrange("p (h d) -> p h d", h=HPC),
                )

        with tc.tile_pool(name="epool", bufs=4) as epool, \
             tc.tile_pool(name="npool", bufs=2) as npool, \
             tc.tile_pool(name="upool", bufs=6) as upool, \
             tc.tile_pool(name="yout", bufs=3) as yout, \
             tc.tile_pool(name="mpool", bufs=1) as mpool, \
             tc.tile_pool(name="pss", bufs=3, space="PSUM") as pss, \
             tc.tile_pool(name="psu", bufs=2, space="PSUM") as psu, \
             tc.tile_pool(name="ypp", bufs=2, space="PSUM") as ypp, \
             tc.tile_pool(name="wop", bufs=1) as wop:

            wo_sb = [single([64, D], f"wo{h}", wop, dt=F32R) for h in range(HPC)]
            for h in range(HPC):
                nc.sync.dma_start(out=wo_sb[h], in_=wo_h[h * 64:(h + 1) * 64, :])

            masks = []
            for off in range(5):
                mt = mpool.tile([128, 512], BF16, tag=f"mask{off}", name=f"mask{off}")
                nc.gpsimd.memset(mt, 1.0)
                nc.gpsimd.affine_select(
                    out=mt,
                    in_=mt,
                    pattern=[[1, 512]],
                    compare_op=mybir.AluOpType.is_ge,
                    fill=0.0,
                    base=1 - off * 128,
                    channel_multiplier=-1,
                )
                masks.append(mt)

            for c in range(NCI):
                s0, s1 = c * 512, (c + 1) * 512
                utn = []
                for h in range(HPC):
                    mc, hp = h // 2, (h % 2) * 64
                    njt = 4 * c + 5
                    ut = psu.tile([65, 512], F32, tag="ut", name="ut")
                    for j in range(njt):
                        sp = pss.tile([128, 512], F32, tag="sp", name="sp")
                        nc.tensor.matmul(
                            sp,
                            kt[mc][hp:hp + 64, j * 128:(j + 1) * 128],
                            qt[mc][hp:hp + 64, s0:s1],
                            start=True,
                            stop=True,
                        )
                        e = epool.tile([128, 512], F32R, tag="e", name="e")
                        nc.scalar.activation(
                            out=e, in_=sp, func=AF.Exp,
                            bias=mb_sb[:, j:j + 1], scale=1.0,
                        )
                        if j >= 4 * c:
                            nc.vector.tensor_mul(
                                out=e, in0=e, in1=masks[j - 4 * c]
                            )
                        nc.tensor.matmul(
                            ut,
                            vsb[j][:, h, :],
                            e,
                            start=(j == 0),
                            stop=(j == njt - 1),
                        )
                    r1_ = npool.tile([1, 512], F32, tag="r1", name="r1")
                    nc.vector.reciprocal(out=r1_, in_=ut[64:65, :])
                    rb = npool.tile([64, 512], F32, tag="rb", name="rb")
                    nc.gpsimd.partition_broadcast(rb, r1_)
                    u = upool.tile([64, 512], F32R, tag="utn", name="utn")
                    nc.vector.tensor_mul(out=u, in0=ut[0:64, :], in1=rb)
                    utn.append(u)
                for dc in range(8):
                    yp = ypp.tile([128, 512], F32, tag="yp", name="yp")
                    for h in range(HPC):
                        nc.tensor.matmul(
                            yp,
                            wo_sb[h][:, dc * 128:(dc + 1) * 128],
                            utn[h],
                            start=(h == 0),
                            stop=(h == HPC - 1),
                        )
                    ysb = yout.tile([128, 512], F32, tag="ysb", name="ysb")
                    nc.vector.tensor_copy(out=ysb, in_=yp)
                    nc.sync.dma_start(
                        out=yt_h[dc * 128:(dc + 1) * 128, s0:s1], in_=ysb
                    )

    nc.compile()
    return nc


def round_f32r(a):
    """RNE-round fp32 to the PE's FP32R format (11-bit mantissa)."""
    b = np.ascontiguousarray(a, dtype=np.float32).view(np.uint32)
    b = (b + np.uint32(0x7FF) + ((b >> np.uint32(12)) & np.uint32(1))) & np.uint32(0xFFFFF000)
    return b.view(np.float32)


def host_inputs(x, mask, freqs, g, Wq, Wkv, Wout, bout, null_kv):
    """Fold g/scale into weights and build the 8 per-core input dicts."""
    f32 = lambda a: np.ascontiguousarray(np.asarray(a, dtype=np.float32))
    x, freqs, g = f32(x), f32(freqs), f32(g)
    Wq, Wkv, Wout = f32(Wq), f32(Wkv), f32(Wout)
    null_kv = f32(null_kv)
    mask = np.asarray(mask, dtype=bool)

    scale = np.float32(DH ** -0.5)
    wq_eff = (Wq * g[:, None]) * scale
    wk_eff = Wkv[:, :H * DH] * g[:, None]
    wv_eff = Wkv[:, H * DH:] * g[:, None]

    cosT = np.ascontiguousarray(np.cos(freqs).T)
    sinT = np.sin(freqs).T.copy()
    sign = np.tile(np.array([-1.0, 1.0], np.float32), DH // 2)
    sinT *= sign[:, None]
    cos2 = np.ascontiguousarray(np.tile(cosT, (2, 1)))
    sin2 = np.ascontiguousarray(np.tile(sinT, (2, 1)))

    mbs = []
    for b in range(B):
        mb = np.full([JPAD], NEG, np.float32)
        mb[0] = 0.0
        mb[1:N + 1] = np.where(mask[b], 0.0, NEG).astype(np.float32)
        mbs.append(mb)

    nk_all = null_kv[0].reshape(H, DH)
    nv_all = null_kv[1].reshape(H, DH)

    in_maps = []
    for core in range(NCORES):
        b, hg = core // 4, core % 4
        h0 = hg * HPC
        in_maps.append({
            "x": np.ascontiguousarray(x[b]),
            "wq": round_f32r(wq_eff[:, h0 * DH:(h0 + HPC) * DH]),
            "wk": round_f32r(wk_eff[:, h0 * DH:(h0 + HPC) * DH]),
            "wv": round_f32r(wv_eff[:, h0 * DH:(h0 + HPC) * DH]),
            "wo": round_f32r(Wout[h0 * DH:(h0 + HPC) * DH, :]),
            "cos2": cos2,
            "sin2": sin2,
            "nk": round_f32r(nk_all[h0:h0 + HPC].reshape(2, 128)),
            "nv": round_f32r(nv_all[h0:h0 + HPC].reshape(1, CPH)),
            "mb": mbs[b],
        })
    return in_maps


_CACHE = {}


def kernel(**inputs):
    if "nc" not in _CACHE:
        _CACHE["nc"] = build_program()
    nc = _CACHE["nc"]

    in_maps = host_inputs(**inputs)

    from concourse.bass_utils import run_bass_kernel_spmd

    res = run_bass_kernel_spmd(
        nc, in_maps, core_ids=list(range(NCORES)), trace=TRACE, **TRACE_KW
    )
    _CACHE["last_result"] = res

    bout = np.asarray(inputs["bout"], dtype=np.float32)
    out = np.empty([B, N, D], np.float32)
    for b in range(B):
        acc = res.results[4 * b]["yt"].astype(np.float32)
        for c in range(4 * b + 1, 4 * b + 4):
            acc = acc + res.results[c]["yt"]
        out[b] = acc.T + bout
    return out
